# revision 1
# baseline (speedup 1.0000x reference)
"""Per-sample batched matmul: out[b,o,f] = sum_i weights[b,o,i] * x[b,i,f].

Sharding: batch (bs=32) split across 8 NeuronCores, 4 samples each, zero
communication. Per sample the kernel PE-transposes W_b (fp32, exact) into
[I, O] layout, then runs accumulating float32r matmuls with x_b as the
moving operand (f32r = fp32 inputs truncated to ~fp22 in the PE, 4x the
fp32 matmul rate; accumulation stays fp32 in PSUM).
"""

import sys

try:  # concourse (Bass/Tile) ships in the container, not on default sys.path
    import concourse  # noqa: F401
except ImportError:
    sys.path.insert(0, "/opt/trn_rl_repo")

import numpy as np

BS, IN_SIZE, OUT_SIZE, FEATS = 32, 1024, 1024, 2048
N_CORES = 8
BPC = BS // N_CORES  # samples per core

P = 128
N_FREE = 512  # moving-operand free dim per matmul (1 PSUM bank of fp32)
KO = IN_SIZE // P  # 8 contraction tiles
MO = OUT_SIZE // P  # 8 output-row tiles
NF = FEATS // N_FREE  # 4 output-col chunks

_NC_CACHE = {}


def _build_nc(mm_dtype_name="float32r"):
    import concourse.mybir as mybir
    import concourse.tile as tile
    from concourse import bacc

    mm_dt = getattr(mybir.dt, mm_dtype_name)

    nc = bacc.Bacc("TRN2", target_bir_lowering=False, debug=False)
    x_d = nc.dram_tensor(
        "x", [BPC, IN_SIZE, FEATS], mybir.dt.float32, kind="ExternalInput"
    ).ap()
    w_d = nc.dram_tensor(
        "w", [BPC, OUT_SIZE, IN_SIZE], mybir.dt.float32, kind="ExternalInput"
    ).ap()
    o_d = nc.dram_tensor(
        "out", [BPC, OUT_SIZE, FEATS], mybir.dt.float32, kind="ExternalOutput"
    ).ap()

    with tile.TileContext(nc) as tc:
        with (
            tc.tile_pool(name="const", bufs=1) as const,
            tc.tile_pool(name="wn_pool", bufs=8) as wn_pool,
            tc.tile_pool(name="wt_pool", bufs=2) as wt_pool,
            tc.tile_pool(name="xn_pool", bufs=4) as xn_pool,
            tc.tile_pool(name="ot_pool", bufs=6) as ot_pool,
            tc.tile_pool(name="psmm", bufs=4, space="PSUM") as psmm_pool,
            tc.tile_pool(name="pstr", bufs=4, space="PSUM") as pstr_pool,
        ):
            # identity via inline DRAM constant: lands by DMA with no
            # engine-serialization in the critical startup path
            eye_d = nc.inline_tensor(np.eye(P, dtype=np.float32), name="eye")
            ident = const.tile([P, P], mm_dt, name="identr")
            nc.sync.dma_start(ident[:], eye_d.ap().bitcast(mm_dt))

            TG = 4  # transposes packed per PSUM bank

            def load_xn(b, n, x_r):
                """x chunk [128, KO, 512], split across 2 DMA queues."""
                xn = xn_pool.tile(
                    [P, KO, N_FREE], mm_dt, tag="xn", name=f"xn_{b}_{n}"
                )
                h = KO // 2
                src = x_r[:, :, n * N_FREE : (n + 1) * N_FREE].bitcast(mm_dt)
                nc.sync.dma_start(xn[:, :h], src[:, :h])
                nc.sync.dma_start(xn[:, h:], src[:, h:])
                return xn

            def transpose_w_block(b, mo, wn, wt):
                """8 f32r transposes of one W row-block; 4 share a PSUM bank,
                leaving via one wide cast-copy, alternating DVE/ACT."""
                for g in range(KO // TG):
                    pt = pstr_pool.tile(
                        [P, TG * P], mm_dt, tag="pt", name=f"pt_{b}_{mo}_{g}"
                    )
                    for c in range(TG):
                        ko = g * TG + c
                        nc.tensor.transpose(
                            pt[:, c * P : (c + 1) * P],
                            wn[:, ko * P : (ko + 1) * P],
                            ident[:],
                        )
                    dst = wt[:, g * TG : (g + 1) * TG, mo, :]
                    srcp = pt[:].rearrange("p (c q) -> p c q", c=TG)
                    if (mo * 2 + g) % 2 == 0:
                        nc.vector.tensor_copy(out=dst, in_=srcp)
                    else:
                        nc.scalar.copy(dst, srcp)

            def load_wn(b, mo, ways=2):
                wn = wn_pool.tile(
                    [P, IN_SIZE], mm_dt, tag="wn", name=f"wn_{b}_{mo}"
                )
                src = w_d[b, mo * P : (mo + 1) * P, :].bitcast(mm_dt)
                w = IN_SIZE // ways
                for q in range(ways):
                    nc.sync.dma_start(
                        wn[:, q * w : (q + 1) * w], src[:, q * w : (q + 1) * w]
                    )
                return wn

            def mm_group(b, n, mo, xn, wt):
                """One [128, 512] output tile: 8 accumulating matmuls,
                DVE psum eviction, output DMA on GpSimd (SWDGE) so its waits
                never head-of-line block input prefetch on Sync."""
                ps = psmm_pool.tile(
                    [P, N_FREE], mybir.dt.float32, tag="ps", name=f"ps_{b}_{n}_{mo}"
                )
                for ko in range(KO):
                    nc.tensor.matmul(
                        ps[:],
                        wt[:, ko, mo, :],
                        xn[:, ko, :],
                        start=(ko == 0),
                        stop=(ko == KO - 1),
                    )
                ot = ot_pool.tile(
                    [P, N_FREE], mybir.dt.float32, tag="ot", name=f"ot_{b}_{n}_{mo}"
                )
                nc.vector.tensor_copy(out=ot[:], in_=ps[:])
                # outputs ride GpSimd (SWDGE): their compute-lagged waits must
                # not share a queue with input prefetch (head-of-line blocking),
                # and Scalar/Sync-issued output DMAs measurably serialize
                # against the kernel-tail drain barrier
                nc.gpsimd.dma_start(
                    o_d[b, mo * P : (mo + 1) * P, n * N_FREE : (n + 1) * N_FREE],
                    ot[:],
                )

            # Ramp HAM (PE clock-gate) off its cold 1.2GHz state with dummy
            # identity transposes during the first W DMA's flight (~5us);
            # the JIT first sample then starts on a warm PE. Tiny DVE sink
            # reads keep them from being treated as dead.
            warm_sink = const.tile([P, 16], mm_dt, name="warm_sink")
            for wg in range(8):
                ptw = pstr_pool.tile([P, TG * P], mm_dt, tag="pt", name=f"ptw_{wg}")
                for c in range(TG):
                    nc.tensor.transpose(
                        ptw[:, c * P : (c + 1) * P], ident[:], ident[:]
                    )
                nc.vector.tensor_copy(out=warm_sink[:], in_=ptw[:, :16])

            for b in range(BPC):
                x_r = x_d[b].rearrange("(ko p) f -> p ko f", p=P)
                wt = wt_pool.tile(
                    [P, KO, MO, P], mm_dt, tag="wt", name=f"wt_{b}"
                )
                if b == 0:
                    # Startup is DMA-limited: interleave each W block's
                    # transposes with its first MM group so the PE paces with
                    # the arriving data instead of waiting for all of W.
                    xn_cur = None
                    for mo in range(MO):
                        wn = load_wn(b, mo, ways=4 if mo < 2 else 2)
                        if mo == 0:
                            xn_cur = load_xn(b, 0, x_r)
                        transpose_w_block(b, mo, wn, wt)
                        mm_group(b, 0, mo, xn_cur, wt)
                        if mo == 4:
                            xn_next = load_xn(b, 1, x_r)
                    start_n = 1
                else:
                    # steady state: x chunk prefetched ahead of the W burst
                    xn_next = load_xn(b, 0, x_r)
                    for mo in range(MO):
                        wn = load_wn(b, mo)
                        transpose_w_block(b, mo, wn, wt)
                    start_n = 0

                for n in range(start_n, NF):
                    xn = xn_next
                    if n + 1 < NF:
                        xn_next = load_xn(b, n + 1, x_r)
                    for mo in range(MO):
                        mm_group(b, n, mo, xn, wt)

    nc.compile()
    return nc


def run(x, weights, trace=False):
    """Shard on batch, run SPMD on 8 cores, gather. Returns (out, results)."""
    from concourse.bass_utils import run_bass_kernel_spmd

    key = "nc"
    if key not in _NC_CACHE:
        _NC_CACHE[key] = _build_nc()
    nc = _NC_CACHE[key]

    x = np.ascontiguousarray(np.asarray(x, dtype=np.float32))
    weights = np.ascontiguousarray(np.asarray(weights, dtype=np.float32))
    in_maps = [
        {
            "x": x[c * BPC : (c + 1) * BPC],
            "w": weights[c * BPC : (c + 1) * BPC],
        }
        for c in range(N_CORES)
    ]
    last_err = None
    for attempt in range(3):
        try:
            res = run_bass_kernel_spmd(
                nc, in_maps, core_ids=list(range(N_CORES)), trace=trace
            )
            break
        except Exception as e:  # transient NRT device faults: back off, retry
            last_err = e
            import time as _time

            _time.sleep(5 * (attempt + 1))
    else:
        raise last_err
    out = np.concatenate([res.results[c]["out"] for c in range(N_CORES)], axis=0)
    return out, res


def kernel(x, weights):
    out, _ = run(x, weights, trace=False)
    return out

